# revision 79
# baseline (speedup 1.0000x reference)
"""MiniBatchDiscrimination Trainium2 kernel (symmetric-halved, v3).

Reference computation:
    m = (x @ T.reshape(512, 1024)).reshape(B, 64, 16)          # [B, out, k]
    norm[i, j, o] = sum_k |m[j, o, k] - m[i, o, k]|
    o_b[i, o] = sum_j exp(-norm[i, j, o]) - 1
    out = concat([x, o_b], axis=1)                             # [B, 576]

Sharding: row-parallel with symmetry halving. Core c receives inputs derived
from x ROTATED by -64c rows, so its 64 rows are rows [0, 64) of its local
view. Row i sums exp(-norm) over the cyclic window j in [i+1, i+256] only
(each unordered pair lands in exactly one window, except distance-256 pairs
which land in two and are corrected separately). Every windowed term
contributes to both endpoint rows: the window-owner's sum accumulates via
the ACT accum_out (dir1), the partner row's contribution accumulates into
local ACC tensors (dir2) that the host rotates back and sums across cores.
The diagonal is never computed, so the reference's "-1" cancels exactly.

The host passes per-core bf16 operands (group-major-packed T, the 320
needed rows of x^T pre-transposed, and TS = sum_k T) — layout/precision
prep so the device streams ~1.4MB instead of 2.6MB and runs no transposes;
the computation pipeline is bf16 throughout either way. The x passthrough
block of the output is assembled on the host directly from the input.

Main loop structure (per core, 64 iterations, ~963ns/iter, PE-bound):
  - per iter: 8 relu tiles relu(m_win - m_i) [128, 256] produced on
    DVE(6)/ACT(every other iter, Relu+bias)/Pool(1); 9 PE matmuls build
    z = 2*sum_k relu - S_win into a per-iter PSUM tile (one -S^T seed with
    an I64 lhsT + 8 k-collapse matmuls with a 2.0-selection lhsT); ACT
    computes exp(-z - S_i) with accum_out -> dir1.
  - dir2: e-tiles added into two separate SBUF accumulators (DVE even
    iters -> bf16 ACCd, Pool odd iters -> f32 ACCp) summed on the host, so
    the chains never serialize; the last iterations all go to DVE to keep
    slow Pool adds off the drain path.
  - per-iter PSUM z tiles (NOT shared pair tiles): a shared tile creates a
    write-after-read serialization between the pair halves that costs ~15%.
  - a PE "warmup" of junk matmuls during the DMA phase holds the tensor
    engine at full clock (p-state) for the projection and early iterations.

Per-core layout:
    partitions p = (o mod 8) * 16 + k   (8 out-features x 16 kernel dims)
    MT[p, g, jj] = m_rot[jj, 8g + (p div 16), p mod 16], g = o div 8
"""

import numpy as np

B, IN_F, OUT_F, K = 512, 512, 64, 16
NCORES = 8
RPC = B // NCORES   # rows per core = 64
NG = OUT_F // 8     # 8 column-groups of 8 out-features x 16 k = 128 partitions
W = 256             # window width
XJ = 320            # j-columns of M needed per core (max col = 63+256 = 319)
ACCW = XJ           # ACC columns: window cols span [1, 320)

_cache = {}


def _build_program(repeat: int = 1, dpool_bufs: int = 26, epool_extra: int = 3,
                   lag: int = 3, act_period: int = 2, z_bufs: int = 4,
                   pm_bufs: int = 3, tail_dve: int = 6, n_warm: int = 10, xt_split: bool = False,
                   seed_rot: bool = False):
    import concourse.bass as bass
    import concourse.bacc as bacc
    import concourse.tile as tile
    from concourse import mybir

    dt = mybir.dt
    f32, bf16 = dt.float32, dt.bfloat16
    Alu = mybir.AluOpType
    Act = mybir.ActivationFunctionType

    nc = bacc.Bacc(num_devices=NCORES)
    t_d = nc.dram_tensor("t", [128, NG * 512], bf16, kind="ExternalInput")
    # xt also carries TS = sum_k T (chunk ft at cols [1280+64ft, 1280+64ft+64)),
    # the same T-collapse the v1 kernel computed on-chip; S^T then comes from
    # four early matmuls that depend only on this one DMA.
    xt_d = nc.dram_tensor("xt", [128, 4 * XJ + 4 * OUT_F], bf16, kind="ExternalInput")
    ob_d = nc.dram_tensor("ob", [OUT_F, RPC], f32, kind="ExternalOutput")
    accd_d = nc.dram_tensor("accd", [OUT_F, ACCW], bf16, kind="ExternalOutput")
    accp_d = nc.dram_tensor("accp", [OUT_F, ACCW], f32, kind="ExternalOutput")
    corr_d = nc.dram_tensor("corr", [OUT_F, RPC], f32, kind="ExternalOutput")

    import ml_dtypes
    from contextlib import ExitStack

    # The slow producers (Pool 450ns, ACT 398ns) take the EARLIEST-completed
    # projection groups so the first iteration isn't gated by slow-relu-on-
    # late-group; DVE's fast relus cover the groups that finish last.
    ACT_G = 1   # relu group computed on ACT (Relu + per-partition bias)
    POOL_G = 0  # relu group computed on Pool
    LAG = lag   # dir2 adds lag the exp by this many iterations

    with tile.TileContext(nc) as tc, ExitStack() as ctx:
        singles = ctx.enter_context(tc.tile_pool(name="singles", bufs=1))

        # One merged constant block, loaded with a single DMA:
        #   cols [0, 120):   ZB — [:, 56-8g : 120-8g] slice is the k-collapse
        #                    lhsT for group g: lhsT_g[p, m] = 2.0 iff m == 8g + p//16
        #   cols [120, 184): I64 (rows 0:64) — the -S^T seed lhsT
        cb_np = np.zeros((128, 184), dtype=ml_dtypes.bfloat16)
        for p in range(128):
            cb_np[p, 56 + p // 16] = 2.0
        for p in range(64):
            cb_np[p, 120 + p] = 1.0
        CB = singles.tile([128, 184], bf16, tag="CB")

        def zb_sl(g):
            return CB[:, 56 - 8 * g : 120 - 8 * g]

        # Persistent operands. T arrives GROUP-MAJOR (host-packed): group g's
        # four 128-row contraction chunks live at cols [512g, 512g+512), so
        # each quarter-DMA completes two whole groups and their projection
        # matmuls fire without waiting for the rest of T. xt is one packed
        # tile with chunk ft at cols [320ft, 320ft+320).
        Tsb = singles.tile([128, NG * 512], bf16, tag="Tsb")
        xT = singles.tile([128, 4 * XJ + 4 * OUT_F], bf16, tag="xT")
        MT = singles.tile([128, NG, XJ], bf16, tag="MT")
        MTf32 = singles.tile([128, NG, RPC], f32, tag="MTf32")  # scalar operand
        negMT6 = singles.tile([128, RPC], f32, tag="negMT6")    # ACT-group bias
        SnegT = singles.tile([OUT_F, XJ], bf16, tag="SnegT")    # -S^T[o, jj]
        SmyNeg = singles.tile([OUT_F, RPC], f32, tag="SmyNeg")  # -S_i[o] (same bf16 rounding)
        # dir2 accumulators: bf16 keeps the DVE adds in the fast 2-byte mode
        # (<=33 adds per column land well inside the 2e-2 tolerance); the
        # Pool one is free to stay f32 (Pool cost is dtype-independent).
        ACCd = singles.tile([OUT_F, ACCW], bf16, tag="ACCd")    # dir2 (DVE)
        ACCp = singles.tile([OUT_F, ACCW], f32, tag="ACCp")     # dir2 (Pool)
        ob_cols = singles.tile([OUT_F, RPC], f32, tag="ob_cols")  # dir1 sums

        nc.vector.memset(ACCd[:, :], 0.0)
        nc.gpsimd.memset(ACCp[:, :], 0.0)

        # ---------------- Prologue: load + project ------------------------
        pps = ctx.enter_context(tc.tile_pool(name="pro_ps", bufs=pm_bufs, space="PSUM"))
        sps = ctx.enter_context(tc.tile_pool(name="s_ps", bufs=1, space="PSUM"))
        zpool = ctx.enter_context(tc.tile_pool(name="zpool", bufs=z_bufs, space="PSUM"))

        # PE p-state warmup: the tensor engine only reaches full clock after
        # ~3us of continuous execution. Junk matmuls over the zeroed ACCd
        # keep it busy through the DMA phase so the real projection (and the
        # first main-loop iterations) run at full rate from the start.
        warm_sink = None
        if n_warm:
            # Warmup lives in a zpool slot (NOT s2's slot, which would stall
            # the S^T matmuls behind the warmup tile's dummy reader).
            wz = zpool.tile([OUT_F, W], f32, tag="z2", name="warm")
            for wi in range(n_warm):
                nc.tensor.matmul(
                    wz[:, :],
                    lhsT=ACCd[0:64, 0:64],
                    rhs=ACCd[0:64, 0:W].bitcast(bf16),
                    start=True,
                    stop=True,
                    skip_group_check=True,
                )
            warm_sink = wz  # read below so the BIR verifier sees a consumer
        # xt's ft0 chunk first as a small DMA so group 0/1 matmuls can fire
        # right after T quarter 0; the rest of xt (+TS) follows T0.
        if xt_split:
            nc.sync.dma_start(out=xT[:, 0:XJ], in_=xt_d[:, 0:XJ])
            nc.scalar.dma_start(out=Tsb[:, 0:1024], in_=t_d[:, 0:1024])
            nc.sync.dma_start(out=xT[:, XJ:], in_=xt_d[:, XJ:])
            t_engs = [nc.scalar, nc.sync, nc.scalar]
            for d in range(1, 4):
                t_engs[d - 1].dma_start(
                    out=Tsb[:, 1024 * d : 1024 * (d + 1)],
                    in_=t_d[:, 1024 * d : 1024 * (d + 1)],
                )
        else:
            nc.sync.dma_start(out=xT[:, :], in_=xt_d[:, :])
            t_engs = [nc.scalar, nc.sync, nc.scalar, nc.sync]
            for d in range(4):
                t_engs[d].dma_start(
                    out=Tsb[:, 1024 * d : 1024 * (d + 1)],
                    in_=t_d[:, 1024 * d : 1024 * (d + 1)],
                )
        # Constants are first needed by the corr/loop matmuls (~10us), so
        # this DMA is issued after the loads it would otherwise delay.
        nc.gpsimd.dma_start(out=CB[:, :], in_=nc.inline_tensor(cb_np, name="cb_c")[:, :])

        # S^T[o, :] = (sum_k T)^T @ x^T from the packed TS chunks — ready as
        # soon as the xt DMA lands, well before the MT chain completes.
        s2 = sps.tile([OUT_F, XJ], f32, tag="s2")
        for ft in range(4):
            nc.tensor.matmul(
                s2[:, :],
                lhsT=xT[:, 4 * XJ + OUT_F * ft : 4 * XJ + OUT_F * (ft + 1)],
                rhs=xT[:, XJ * ft : XJ * (ft + 1)],
                start=(ft == 0),
                stop=(ft == 3),
            )
        nc.scalar.mul(SnegT[:, :], s2[:, :], -1.0)
        nc.vector.tensor_copy(out=SmyNeg[:, :], in_=SnegT[:, 0:RPC])

        # MT[p, g, :] = (T_group_g)^T @ x^T
        # GPSIMD cannot read PSUM, so the pm->MT copies alternate ACT/DVE.
        mt_cp = [nc.scalar, nc.vector, nc.scalar, nc.vector,
                 nc.scalar, nc.vector, nc.scalar, nc.scalar]

        def copy_on(eng, out, in_):
            if eng is nc.scalar:
                eng.copy(out=out, in_=in_)
            else:
                eng.tensor_copy(out=out, in_=in_)

        for g in range(NG):
            pm = pps.tile([128, XJ], f32, tag="pm", name=f"pm{g}")
            for ft in range(4):
                nc.tensor.matmul(
                    pm[:, :],
                    lhsT=Tsb[:, 512 * g + 128 * ft : 512 * g + 128 * (ft + 1)],
                    rhs=xT[:, XJ * ft : XJ * (ft + 1)],
                    start=(ft == 0),
                    stop=(ft == 3),
                )
            copy_on(mt_cp[g], MT[:, g, :], pm[:, :])
            nc.vector.tensor_copy(out=MTf32[:, g, :], in_=MT[:, g, 0:RPC])
        nc.scalar.mul(negMT6[:, :], MT[:, ACT_G, 0:RPC], -1.0)

        # ---------------- Main loop over this core's 64 rows --------------
        dpool = ctx.enter_context(tc.tile_pool(name="dpool", bufs=dpool_bufs))
        cpool = ctx.enter_context(tc.tile_pool(name="cpool", bufs=4))
        epool = ctx.enter_context(tc.tile_pool(name="epool", bufs=LAG + epool_extra))

        # ------------- distance-256 correction pairs (qq, qq+256) ---------
        # corr_step emits one piece of the correction chain; all seven run
        # before the main loop (anything that lets corr execute during the
        # loop regresses ~6.5us and mid-loop emission also miscomputes).
        corr_sb = singles.tile([OUT_F, RPC], f32, tag="corr_sb")
        if warm_sink is not None:
            # Dummy read of the warmup tile (overwritten by the corr exp).
            nc.scalar.copy(out=corr_sb[:, 0:1], in_=warm_sink[0:64, 0:1])
        corr_state = {}

        def corr_step(step):
            cs = corr_state
            if step == 0:
                cs["d0"] = cpool.tile([128, NG, RPC], bf16, tag="cd", name="d0")
                nc.vector.tensor_sub(
                    cs["d0"][:, :, :], MT[:, :, 0:RPC], MT[:, :, W : W + RPC]
                )
            elif step == 1:
                cs["r2"] = cpool.tile([128, NG, RPC], bf16, tag="cd", name="r2")
                nc.vector.tensor_scalar(
                    cs["r2"][:, :, :], cs["d0"][:, :, :], -1.0, 0.0,
                    Alu.mult, Alu.max,
                )
            elif step == 2:
                cs["r1"] = cpool.tile([128, NG, RPC], bf16, tag="cd", name="r1")
                nc.vector.tensor_relu(cs["r1"][:, :, :], cs["d0"][:, :, :])
            elif step == 3:
                pass  # |d0| never materialized: z3 collapses r1 and r2
            elif step == 4:
                # 16 small matmuls instead of an extra DVE add pass: the PE
                # is idle here while DVE gates the first loop iteration.
                cs["z3"] = zpool.tile([OUT_F, RPC], f32, tag="z2", name="z3")
                for half in range(2):
                    rr = cs["r1"] if half == 0 else cs["r2"]
                    for g in range(NG):
                        nc.tensor.matmul(
                            cs["z3"][:, :],
                            lhsT=zb_sl(g),
                            rhs=rr[:, g, :],
                            start=(half == 0 and g == 0),
                            stop=(half == 1 and g == NG - 1),
                        )
            elif step == 5:
                nc.scalar.activation(
                    out=corr_sb[:, :], in_=cs["z3"][:, :], func=Act.Exp,
                    scale=-0.5,
                )
            elif step == 6:
                nc.sync.dma_start(out=corr_d[:, :], in_=corr_sb[:, :])

        # ---------------- Main loop ---------------------------------------
        e_hist = []
        n_it = RPC * repeat

        def flush_dir2(n_keep):
            while len(e_hist) > n_keep:
                li, le = e_hist.pop(0)
                llo = li % RPC + 1
                # The last few iterations' adds all go to DVE (127ns vs
                # Pool's 603ns) so the drain after the final exp is short.
                if li % 2 == 0 or li >= n_it - tail_dve:
                    nc.vector.tensor_add(
                        ACCd[:, llo : llo + W], ACCd[:, llo : llo + W], le[:, :]
                    )
                else:
                    nc.gpsimd.tensor_add(
                        ACCp[:, llo : llo + W], ACCp[:, llo : llo + W], le[:, :]
                    )

        def produce(i):
            # Emit the 8 relu tiles for iteration i. The ACT-owned group
            # alternates to DVE on odd iterations so ACT stays under the
            # PE-bound iteration budget.
            lo = i % RPC + 1
            tiles = []
            for g in range(NG):
                r_g = dpool.tile([128, W], bf16, tag="d")
                # Odd window offsets are fine for the DVE fast mode here
                # (unit-stride 2-byte APs; no aligned shifted copy needed).
                win = MT[:, g, lo : lo + W]
                if g == ACT_G and (
                    (i % 3 == 0) if seed_rot else (i % act_period == 0)
                ):
                    nc.scalar.activation(
                        out=r_g[:, :],
                        in_=win,
                        func=Act.Relu,
                        scale=1.0,
                        bias=negMT6[:, i : i + 1],
                    )
                elif g == ACT_G and seed_rot and i % 3 == 1:
                    # g6 on Pool this iteration (ACT does the z seed instead)
                    nc.gpsimd.tensor_scalar(
                        r_g[:, :], win, MTf32[:, g, i : i + 1], 0.0,
                        Alu.subtract, Alu.max,
                    )
                elif g == POOL_G:
                    nc.gpsimd.tensor_scalar(
                        r_g[:, :], win, MTf32[:, g, i : i + 1], 0.0,
                        Alu.subtract, Alu.max,
                    )
                else:
                    nc.vector.tensor_scalar(
                        r_g[:, :], win, MTf32[:, g, i : i + 1], 0.0,
                        Alu.subtract, Alu.max,
                    )
                tiles.append(r_g)
            return tiles

        # corr is emitted before the loop; variants that let it execute
        # during the loop (mid-loop emission, deprioritized scheduling, or
        # emitting the first produce batches ahead of it) all regress.
        for _cstep in range(7):
            corr_step(_cstep)

        r_cur = produce(0)
        for it_idx in range(n_it):
            i = it_idx % RPC
            lo = i + 1  # window = [lo, lo + W)
            # Software pipelining: the next iteration's tiles are emitted
            # before this iteration's collapse/exp so the producer engines
            # never sit behind the PE->ACT dependency chain.
            r_fut = produce((it_idx + 1) % RPC) if it_idx + 1 < n_it else None

            z = zpool.tile([OUT_F, W], f32, tag="z2")
            act_seed = seed_rot and i % 3 == 1
            if act_seed:
                # BROKEN on this stack: PE start=False accumulation does not
                # compose with a compute-engine PSUM write (the accumulator
                # state machine ignores it and stop overwrites the seed).
                # Kept only as a record; seed_rot must stay False.
                nc.scalar.copy(out=z[:, :], in_=SnegT[:, lo : lo + W])
            else:
                nc.tensor.matmul(
                    z[:, :],
                    lhsT=CB[0:64, 120:184],
                    rhs=SnegT[:, lo : lo + W],
                    start=True,
                    stop=False,
                )
            for g in range(NG):
                nc.tensor.matmul(
                    z[:, :],
                    lhsT=zb_sl(g),
                    rhs=r_cur[g][:, :],
                    start=False,
                    stop=(g == NG - 1),
                    skip_group_check=act_seed,
                )
            e = epool.tile([OUT_F, W], bf16, tag="e")
            nc.scalar.activation(
                out=e[:, :],
                in_=z[:, :],
                func=Act.Exp,
                scale=-1.0,
                bias=SmyNeg[:, i : i + 1],
                accum_out=ob_cols[:, i : i + 1],
            )
            e_hist.append((it_idx, e))
            flush_dir2(LAG)
            r_cur = r_fut
        flush_dir2(0)


        # ---------------- Epilogue: stores -------------------------------
        # dir1 ships column-major (the 64x64 transpose happens on the host):
        # its DMA stages the moment the last exp's accumulate lands, instead
        # of queueing transposes behind the final dir2 add on DVE.
        nc.sync.dma_start(out=ob_d[:, :], in_=ob_cols[:, :])
        nc.gpsimd.dma_start(out=accd_d[:, :], in_=ACCd[:, :])
        nc.sync.dma_start(out=accp_d[:, :], in_=ACCp[:, :])

    nc.compile()
    if not nc.is_finalized():
        nc.finalize()
    return nc


def _get_program():
    if "nc" not in _cache:
        _cache["nc"] = _build_program()
    return _cache["nc"]


def kernel(x: np.ndarray, T: np.ndarray) -> np.ndarray:
    import os

    import ml_dtypes

    from concourse.bass_utils import run_bass_kernel_spmd

    nc = _get_program()
    x = np.ascontiguousarray(x, dtype=np.float32)
    t2 = np.ascontiguousarray(T, dtype=np.float32).reshape(IN_F, OUT_F * K)
    t_bf = t2.astype(ml_dtypes.bfloat16)
    # Group-major packing: tg[p, 512g + 128ft + c] = T[128ft + p, 128g + c],
    # so each quarter of the tg DMA delivers two complete groups.
    tg = np.empty((128, NG * 512), dtype=ml_dtypes.bfloat16)
    for g in range(NG):
        for ft in range(4):
            tg[:, 512 * g + 128 * ft : 512 * g + 128 * (ft + 1)] = t_bf[
                128 * ft : 128 * (ft + 1), 128 * g : 128 * (g + 1)
            ]
    tg = np.ascontiguousarray(tg)
    ts_bf = (
        t_bf.astype(np.float32)
        .reshape(IN_F, OUT_F, K)
        .sum(axis=2)
        .astype(ml_dtypes.bfloat16)
    )  # [IN_F, OUT_F] = sum_k T, as v1 computed on-chip
    in_maps = []
    for c in range(NCORES):
        xr = np.roll(x, -RPC * c, axis=0)
        xtt = xr[0:XJ, :].T.astype(ml_dtypes.bfloat16)  # [IN_F, XJ]
        xp = np.empty((128, 4 * XJ + 4 * OUT_F), dtype=ml_dtypes.bfloat16)
        for ft in range(4):
            xp[:, XJ * ft : XJ * (ft + 1)] = xtt[128 * ft : 128 * (ft + 1), :]
            xp[:, 4 * XJ + OUT_F * ft : 4 * XJ + OUT_F * (ft + 1)] = ts_bf[
                128 * ft : 128 * (ft + 1), :
            ]
        in_maps.append({"xt": np.ascontiguousarray(xp), "t": tg})
    try:
        res = run_bass_kernel_spmd(nc, in_maps, core_ids=list(range(NCORES)))
    except ModuleNotFoundError:
        # BASS_TRACE requested but the axon NTFF hook (antenv) is absent in
        # this container — retry with tracing disabled.
        os.environ["BASS_NEVER_TRACE"] = "1"
        res = run_bass_kernel_spmd(nc, in_maps, core_ids=list(range(NCORES)))
    _cache["last_results"] = res

    out_full = np.empty((B, IN_F + OUT_F), np.float32)
    out_full[:, :IN_F] = x                                         # passthrough
    ob = np.zeros((B, OUT_F), np.float64)
    for c in range(NCORES):
        r = res.results[c]
        ob[RPC * c : RPC * (c + 1)] += np.asarray(r["ob"]).T       # dir1
        tmp = np.zeros((OUT_F, B), np.float64)
        tmp[:, :ACCW] = np.asarray(r["accd"], np.float64) + np.asarray(
            r["accp"], np.float64
        )
        ob += np.roll(tmp, RPC * c, axis=1).T                      # dir2
    for c in range(4):  # distance-256 corrections, canonical q in [0, 256)
        corr = np.asarray(res.results[c]["corr"], np.float64).T    # [RPC, OUT_F]
        ob[RPC * c : RPC * (c + 1)] -= corr
        ob[RPC * c + W : RPC * (c + 1) + W] -= corr
    out_full[:, IN_F:] = ob.astype(np.float32)
    return out_full


if __name__ == "__main__":
    rng = np.random.default_rng(0)
    x = rng.standard_normal((B, IN_F), dtype=np.float32)
    T = rng.standard_normal((IN_F, OUT_F, K), dtype=np.float32)
    out = kernel(x, T)
    print("out shape:", out.shape, out.dtype)
    print("x passthrough exact:", np.array_equal(out[:, :IN_F], x))
    print("o_b stats:", np.abs(out[:, IN_F:]).max())
